# revision 20
# baseline (speedup 1.0000x reference)
"""CombPool2d Trainium2 kernel (fp16 I/O).

out = (w_avg**2) * avg_pool2x2(x) + (w_max**2) * max_pool2x2(x)
x: (16, 192, 224, 224) f32, w_avg/w_max: (1, 192, 1, 1) f32.

Sharding: data-parallel over batch — 2 batches per NeuronCore on 8 cores.

The kernel is DMA-bandwidth-bound (exclusive DMA-engine pool, ~360 GB/s
aggregate), so I/O rides in fp16: the host casts x to fp16 (rel-err
~2.4e-4, far inside the 2e-2 gate) and upcasts the fp16 result back to
f32.  Per-core traffic drops 96.3 MB -> 48.2 MB, i.e. a ~134 us floor
instead of ~268 us.

Layout trick (unchanged from the f32 version): flatten (C, H) so each
output row (one (c, j) pair, 112 output pixels) is produced from 448
contiguous input floats.  Per batch there are 192*112 = 21504 row-pairs,
tiled as (128 partitions x krp row-pairs); every DMA is a fully
contiguous HBM read/write with >=512 B per-partition lines.

Engine assignment is chosen for the fp16 cost model (DVE gets a 2x mode
only for 2-byte dtypes with packed innermost access; ACT and Pool are
dtype-blind; the GPSIMD/Pool backend in this walrus implements only
add/multiply, so every max must run on DVE):

  rs = row0 + row1    (DVE, contiguous, 2x)
  rm = max(row0,row1) (DVE, contiguous, 2x)
  cs = rs_e + rs_o    (Pool/GPSIMD, stride-2)
  cm = max(rm_e,rm_o) (DVE, stride-2, 1x)
  csx = cs * (wavg2/4)  (ACT, per-partition f32 scalar)
  cmx = cm * wmax2      (ACT, per-partition f32 scalar)
  out = csx + cmx     (DVE 2x; every 3rd tile on Pool for balance)

DVE is the compute bottleneck (~135us busy vs the ~134us DMA floor);
timeline-sim/HW: 154.2us vs 267.6us for the f32 version's HBM roofline.

The sum association matches XLA's (a+b)+(c+d) up to fp16 rounding.
Input DMAs ride the SP HWDGE ring, output DMAs the ACT ring.  Channel
coefficients are per-partition scalars (krp divides 112), precomputed on
host and DMA'd once as f32 (scalar operands are exempt from the 2-byte
rule).
"""

import json

import numpy as np

import concourse.bass as bass
import concourse.mybir as mybir
from concourse.tile import TileContext
from concourse.bass_utils import run_bass_kernel_spmd


def _split_multi_waits(bir: dict) -> dict:
    """The walrus build in this container rejects instructions carrying more
    than one semaphore wait ("Too many sync wait commands").  Engines execute
    their instruction stream in order, so hoisting all-but-one wait onto
    standalone EventSemaphore instructions inserted immediately before the
    instruction is semantically identical."""
    ctr = 0
    for fn in bir["functions"]:
        for blk in fn["blocks"]:
            out = []
            for ins in blk["instructions"]:
                si = ins.get("sync_info")
                waits = si.get("on_wait", []) if si else []
                if len(waits) > 1:
                    for w in waits[:-1]:
                        ctr += 1
                        out.append(
                            {
                                "debug": ins.get("debug", 0),
                                "engine": ins["engine"],
                                "ins": [],
                                "outs": [],
                                "name": f"{ins['name']}-sw{ctr}",
                                "opcode": "EventSemaphore",
                                "sync_info": {"on_update": [], "on_wait": [w]},
                            }
                        )
                    si["on_wait"] = [waits[-1]]
                out.append(ins)
            blk["instructions"] = out
    return bir


def _strip_dead_const_memsets(bir: dict) -> dict:
    """Drop the framework's const-AP memsets when nothing reads them (this
    kernel uses no activation-table constants).  They run on Pool ahead of
    the entry barrier and delay everyone's start."""
    read = set()
    for fn in bir["functions"]:
        for blk in fn["blocks"]:
            for ins in blk["instructions"]:
                for arg in ins.get("ins", []):
                    if isinstance(arg, dict):
                        read.add(arg.get("memref"))
    for fn in bir["functions"]:
        for blk in fn["blocks"]:
            blk["instructions"] = [
                ins
                for ins in blk["instructions"]
                if not (
                    ins.get("opcode") == "Memset"
                    and str(
                        (ins.get("outs") or [{}])[0].get("memref", "")
                    ).startswith("const-")
                    and (ins.get("outs") or [{}])[0].get("memref") not in read
                    and not (ins.get("sync_info") or {}).get("on_wait")
                    and not (ins.get("sync_info") or {}).get("on_update")
                )
            ]
    return bir


class _SplitWaitsBass(bass.Bass):
    def to_json_bytes(self) -> bytes:
        d = json.loads(super().to_json_bytes())
        _strip_dead_const_memsets(d)
        _split_multi_waits(d)
        return json.dumps(d).encode()


B, C, H, W = 16, 192, 224, 224
OH, OW = H // 2, W // 2
NCORES = 8
BPC = B // NCORES              # batches per core
P = 128                        # SBUF partitions

_nc_cache = []


def build_variant(
    krp=14,
    xbufs=6,
    rbufs=4,
    obufs=6,
    comb_pool_mod=2,
    comb_defer=False,
    rs_pool_mod=0,
    tail_cfg=None,
    hold_stores=0,
    hold_wait_ms=0.105,
):
    """The GPSIMD (Pool) backend in this walrus only implements add/multiply
    ops, so every max runs on DVE; Pool gets the strided column add and, for
    every comb_pool_mod-th tile, the final combine add (load balance: DVE is
    otherwise the bottleneck at ~141us vs the ~134us DMA floor).

    tail_cfg: list of per-tile specs applied to the LAST len(tail_cfg) tiles
    (in order), engineering the pipeline drain: each spec is a dict with
      plan:  tuple of row-pair piece sizes (sum == krp, each >= 3)
      split: bool — split the input DMA per piece
      eng:   per-piece (cs_engine, combine_engine) chars, 'v'=DVE, 'g'=Pool
      fuse:  per-piece bool — fuse coefA-scale+combine into one DVE
             scalar_tensor_tensor (one ACT mul instead of two, one less hop)
    The goal is that no engine holds a backlog when the last input piece
    lands: the final pieces' chains then drain every engine in parallel.

    hold_stores: the first hold_stores tiles' output stores are deferred
    (scheduler wait-until hint) so they drain the exclusive DMA engines
    while the last tiles compute."""
    if tail_cfg is None:
        tail_cfg = []
    f16 = mybir.dt.float16
    f32 = mybir.dt.float32
    tpb = (C * OH) // (P * krp)
    nt = BPC * tpb
    fin = krp * 2 * W
    fout = krp * OW
    assert 112 % krp == 0 and (C * OH) % (P * krp) == 0
    for spec in tail_cfg:
        assert sum(spec["plan"]) == krp and all(s >= 3 for s in spec["plan"])
        assert len(spec["eng"]) == len(spec["plan"])

    nc = _SplitWaitsBass()
    x_d = nc.dram_tensor("x", [nt, P, fin], f16, kind="ExternalInput")
    coef_d = nc.dram_tensor("coef", [P, 2 * tpb], f32, kind="ExternalInput")
    out_d = nc.dram_tensor("out", [nt, P, fout], f16, kind="ExternalOutput")

    with TileContext(nc) as tc:
        with (
            tc.tile_pool(name="cpool", bufs=1) as cpool,
            tc.tile_pool(name="xpool", bufs=xbufs) as xpool,
            tc.tile_pool(name="rpool", bufs=rbufs) as rpool,
            tc.tile_pool(name="opool", bufs=obufs) as opool,
            tc.tile_pool(name="hpool", bufs=max(hold_stores, 1)) as hpool,
        ):
            held = []
            pending = []  # deferred Pool combines (emitted after next cs)
            # First x tile load is issued before the coef load so the SP DMA
            # ring starts on the big transfer immediately; coef rides the ACT
            # ring.  Multi-sem waits on the consumers are handled by the
            # _SplitWaitsBass serializer.
            xt0 = xpool.tile([P, fin], f16, tag="xt", name="xt0")
            nc.sync.dma_start(xt0, x_d[0])
            coef = cpool.tile([P, 2 * tpb], f32)
            nc.scalar.dma_start(coef, coef_d[:, :])
            coefA = coef[:, :tpb]
            coefM = coef[:, tpb:]
            for i in range(nt):
                tb = i % tpb
                spec = (
                    tail_cfg[i - (nt - len(tail_cfg))]
                    if i >= nt - len(tail_cfg)
                    else None
                )
                plan = spec["plan"] if spec else (krp,)
                split_in = bool(spec and spec.get("split"))

                if i == 0:
                    xt = xt0
                elif not split_in:
                    xt = xpool.tile([P, fin], f16, tag="xt")
                    nc.sync.dma_start(xt, x_d[i])
                else:
                    xt = xpool.tile([P, fin], f16, tag="xt")
                    off = 0
                    for seg in plan:
                        fi = seg * 2 * W
                        nc.sync.dma_start(
                            xt[:, off * 2 * W : off * 2 * W + fi],
                            x_d[i][:, off * 2 * W : off * 2 * W + fi],
                        )
                        off += seg
                x4 = xt.rearrange("p (s two w) -> p s two w", two=2, w=W)

                off = 0
                for pi, seg in enumerate(plan):
                    sl = slice(off, off + seg)
                    fo = seg * OW
                    ostart = off * OW
                    off += seg
                    if spec:
                        cs_c, cb_c = spec["eng"][pi]
                        fuse = spec.get("fuse", (False,) * len(plan))[pi]
                    else:
                        cs_c = "g"
                        cb_c = (
                            "g"
                            if comb_pool_mod
                            and i % comb_pool_mod == comb_pool_mod - 1
                            else "v"
                        )
                        fuse = False
                    eng = {"v": nc.vector, "g": nc.gpsimd}

                    # Row stage: full-width contiguous fp16 ops -> DVE 2x.
                    # (every rs_pool_mod-th tile's rs runs on Pool: it feeds
                    # cs on the same engine, so no cross-engine hop is added)
                    rs_eng = (
                        nc.gpsimd
                        if rs_pool_mod
                        and not spec
                        and i % rs_pool_mod == rs_pool_mod - 1
                        else nc.vector
                    )
                    rs = rpool.tile([P, seg * W], f16, tag="rs")
                    rs_eng.tensor_add(
                        rs.rearrange("p (s w) -> p s w", w=W),
                        x4[:, sl, 0, :],
                        x4[:, sl, 1, :],
                    )
                    rm = rpool.tile([P, seg * W], f16, tag="rm")
                    nc.vector.tensor_max(
                        rm.rearrange("p (s w) -> p s w", w=W),
                        x4[:, sl, 0, :],
                        x4[:, sl, 1, :],
                    )

                    # Column stage: stride-2 ops, engine per config.
                    rs4 = rs.rearrange("p (s w two) -> p s w two", two=2, w=OW)
                    cs = rpool.tile([P, fo], f16, tag="cs")
                    eng[cs_c].tensor_add(
                        cs.rearrange("p (s w) -> p s w", w=OW),
                        rs4[:, :, :, 0],
                        rs4[:, :, :, 1],
                    )
                    for fn_ in pending:
                        fn_()
                    pending.clear()
                    rm4 = rm.rearrange("p (s w two) -> p s w two", two=2, w=OW)
                    cm = rpool.tile([P, fo], f16, tag="cm")
                    nc.vector.tensor_max(
                        cm.rearrange("p (s w) -> p s w", w=OW),
                        rm4[:, :, :, 0],
                        rm4[:, :, :, 1],
                    )

                    # Scale + combine.  Normal: both scalings on ACT (in
                    # place), contiguous fp16 add on DVE (2x).  fuse: one ACT
                    # scaling + fused DVE scalar_tensor_tensor (one less
                    # dependency hop on the drain path).
                    hold = i < hold_stores
                    pool_ = hpool if hold else opool
                    ot = pool_.tile([P, fo], f16, tag="oth" if hold else "ot")
                    if fuse:
                        nc.scalar.mul(cm, cm, coefM[:, tb : tb + 1])
                        nc.vector.scalar_tensor_tensor(
                            ot,
                            cs,
                            coefA[:, tb : tb + 1],
                            cm,
                            op0=mybir.AluOpType.mult,
                            op1=mybir.AluOpType.add,
                        )
                    else:
                        nc.scalar.mul(cs, cs, coefA[:, tb : tb + 1])
                        nc.scalar.mul(cm, cm, coefM[:, tb : tb + 1])
                        if cb_c == "g" and comb_defer and not spec:
                            dst = out_d[i][:, ostart : ostart + fo]

                            def emit(ot=ot, cs=cs, cm=cm, dst=dst, hold=hold):
                                nc.gpsimd.tensor_add(ot, cs, cm)
                                if hold:
                                    held.append((dst, ot))
                                else:
                                    nc.scalar.dma_start(dst, ot)

                            pending.append(emit)
                            continue
                        eng[cb_c].tensor_add(ot, cs, cm)
                    if hold:
                        held.append((out_d[i][:, ostart : ostart + fo], ot))
                    else:
                        nc.scalar.dma_start(
                            out_d[i][:, ostart : ostart + fo], ot
                        )
            for fn_ in pending:
                fn_()
            pending.clear()
            # Deferred stores: scheduled (via the wait-until hint, which is
            # scheduler-only and not serialized) to land on the ACT ring just
            # after the input stream ends, so they drain the exclusive DMA
            # engines while the last tiles compute.
            for j, (dst, ot) in enumerate(held):
                with tc.tile_wait_until(hold_wait_ms + j * 1e-4):
                    nc.scalar.dma_start(dst, ot)
    nc._variant = dict(krp=krp, tpb=tpb, nt=nt, fin=fin, fout=fout)
    return nc


# current best configuration used by kernel()
BEST = dict(
    krp=14, xbufs=6, rbufs=4, obufs=6, comb_pool_mod=3, hold_stores=0,
    tail_cfg=[
        dict(
            plan=(8, 3, 3),
            split=True,
            eng=[("g", "g"), ("v", "v"), ("v", "v")],
            fuse=(False, True, True),
        )
    ],
)


def get_nc():
    if not _nc_cache:
        _nc_cache.append(build_variant(**BEST))
    return _nc_cache[0]


def make_coef(w_avg, w_max, krp, tpb):
    # Coefficients stay f32 (scalar operands are exempt from the fp16 rule);
    # (w*w)/4 is an exact exponent shift in fp32.
    wa = np.asarray(w_avg).reshape(C).astype(np.float32)
    wm = np.asarray(w_max).reshape(C).astype(np.float32)
    ca = (wa * wa) / np.float32(4.0)
    cm = wm * wm
    # partition p of tile tb covers channel (tb*P*krp + p*krp) // OH
    chan = (
        np.arange(tpb)[None, :] * P * krp + np.arange(P)[:, None] * krp
    ) // OH  # (P, tpb)
    return np.concatenate([ca[chan], cm[chan]], axis=1).astype(np.float32)


def make_in_maps(x, w_avg, w_max, v):
    coef = make_coef(w_avg, w_max, v["krp"], v["tpb"])
    x = np.asarray(x).astype(np.float16)
    in_maps = []
    for c in range(NCORES):
        xc = np.ascontiguousarray(x[c * BPC : (c + 1) * BPC]).reshape(
            v["nt"], P, v["fin"]
        )
        in_maps.append({"x": xc, "coef": coef})
    return in_maps


def kernel(x, w_avg, w_max):
    nc = get_nc()
    in_maps = make_in_maps(x, w_avg, w_max, nc._variant)
    try:
        res = run_bass_kernel_spmd(nc, in_maps, core_ids=list(range(NCORES)))
    except Exception:
        # A previously-crashed run can leave the device wedged; one retry
        # after it resets is usually enough.
        import time

        time.sleep(5)
        res = run_bass_kernel_spmd(nc, in_maps, core_ids=list(range(NCORES)))
    outs = [
        r["out"].astype(np.float32).reshape(BPC, C, OH, OW) for r in res.results
    ]
    return np.concatenate(outs, axis=0)


# revision 22
# speedup vs baseline: 1.0025x; 1.0025x over previous
"""CombPool2d Trainium2 kernel (fp16 I/O).

out = (w_avg**2) * avg_pool2x2(x) + (w_max**2) * max_pool2x2(x)
x: (16, 192, 224, 224) f32, w_avg/w_max: (1, 192, 1, 1) f32.

Sharding: data-parallel over batch — 2 batches per NeuronCore on 8 cores.

The kernel is DMA-bandwidth-bound (exclusive DMA-engine pool, ~360 GB/s
aggregate), so I/O rides in fp16: the host casts x to fp16 (rel-err
~2.4e-4, far inside the 2e-2 gate) and upcasts the fp16 result back to
f32.  Per-core traffic drops 96.3 MB -> 48.2 MB, i.e. a ~134 us floor
instead of ~268 us.

Layout trick (unchanged from the f32 version): flatten (C, H) so each
output row (one (c, j) pair, 112 output pixels) is produced from 448
contiguous input floats.  Per batch there are 192*112 = 21504 row-pairs,
tiled as (128 partitions x krp row-pairs); every DMA is a fully
contiguous HBM read/write with >=512 B per-partition lines.

Engine assignment is chosen for the fp16 cost model (DVE gets a 2x mode
only for 2-byte dtypes with packed innermost access; ACT and Pool are
dtype-blind; the GPSIMD/Pool backend in this walrus implements only
add/multiply, so every max must run on DVE):

  rs = row0 + row1    (DVE, contiguous, 2x)
  rm = max(row0,row1) (DVE, contiguous, 2x)
  cs = rs_e + rs_o    (Pool/GPSIMD, stride-2)
  cm = max(rm_e,rm_o) (DVE, stride-2, 1x)
  csx = cs * (wavg2/4)  (ACT, per-partition f32 scalar)
  cmx = cm * wmax2      (ACT, per-partition f32 scalar)
  out = csx + cmx     (DVE 2x; every 3rd tile on Pool for balance)

DVE is the compute bottleneck (~135us busy vs the ~134us DMA floor);
timeline-sim/HW: 154.2us vs 267.6us for the f32 version's HBM roofline.

The sum association matches XLA's (a+b)+(c+d) up to fp16 rounding.
Input DMAs ride the SP HWDGE ring, output DMAs the ACT ring.  Channel
coefficients are per-partition scalars (krp divides 112), precomputed on
host and DMA'd once as f32 (scalar operands are exempt from the 2-byte
rule).
"""

import json

import numpy as np

import concourse.bass as bass
import concourse.mybir as mybir
from concourse.tile import TileContext
from concourse.bass_utils import run_bass_kernel_spmd


def _split_multi_waits(bir: dict) -> dict:
    """The walrus build in this container rejects instructions carrying more
    than one semaphore wait ("Too many sync wait commands").  Engines execute
    their instruction stream in order, so hoisting all-but-one wait onto
    standalone EventSemaphore instructions inserted immediately before the
    instruction is semantically identical."""
    ctr = 0
    for fn in bir["functions"]:
        for blk in fn["blocks"]:
            out = []
            for ins in blk["instructions"]:
                si = ins.get("sync_info")
                waits = si.get("on_wait", []) if si else []
                if len(waits) > 1:
                    for w in waits[:-1]:
                        ctr += 1
                        out.append(
                            {
                                "debug": ins.get("debug", 0),
                                "engine": ins["engine"],
                                "ins": [],
                                "outs": [],
                                "name": f"{ins['name']}-sw{ctr}",
                                "opcode": "EventSemaphore",
                                "sync_info": {"on_update": [], "on_wait": [w]},
                            }
                        )
                    si["on_wait"] = [waits[-1]]
                out.append(ins)
            blk["instructions"] = out
    return bir


def _strip_dead_const_memsets(bir: dict) -> dict:
    """Drop the framework's const-AP memsets when nothing reads them (this
    kernel uses no activation-table constants).  They run on Pool ahead of
    the entry barrier and delay everyone's start."""
    read = set()
    for fn in bir["functions"]:
        for blk in fn["blocks"]:
            for ins in blk["instructions"]:
                for arg in ins.get("ins", []):
                    if isinstance(arg, dict):
                        read.add(arg.get("memref"))
    for fn in bir["functions"]:
        for blk in fn["blocks"]:
            blk["instructions"] = [
                ins
                for ins in blk["instructions"]
                if not (
                    ins.get("opcode") == "Memset"
                    and str(
                        (ins.get("outs") or [{}])[0].get("memref", "")
                    ).startswith("const-")
                    and (ins.get("outs") or [{}])[0].get("memref") not in read
                    and not (ins.get("sync_info") or {}).get("on_wait")
                    and not (ins.get("sync_info") or {}).get("on_update")
                )
            ]
    return bir


class _SplitWaitsBass(bass.Bass):
    def to_json_bytes(self) -> bytes:
        d = json.loads(super().to_json_bytes())
        _strip_dead_const_memsets(d)
        _split_multi_waits(d)
        return json.dumps(d).encode()


B, C, H, W = 16, 192, 224, 224
OH, OW = H // 2, W // 2
NCORES = 8
BPC = B // NCORES              # batches per core
P = 128                        # SBUF partitions

_nc_cache = []


def build_variant(
    krp=14,
    xbufs=6,
    rbufs=4,
    obufs=6,
    comb_pool_mod=2,
    comb_defer=False,
    rs_pool_mod=0,
    head_cfg=None,
    tail_cfg=None,
    hold_stores=0,
    hold_wait_ms=0.105,
):
    """The GPSIMD (Pool) backend in this walrus only implements add/multiply
    ops, so every max runs on DVE; Pool gets the strided column add and, for
    every comb_pool_mod-th tile, the final combine add (load balance: DVE is
    otherwise the bottleneck at ~141us vs the ~134us DMA floor).

    tail_cfg: list of per-tile specs applied to the LAST len(tail_cfg) tiles
    (in order), engineering the pipeline drain: each spec is a dict with
      plan:  tuple of row-pair piece sizes (sum == krp, each >= 3)
      split: bool — split the input DMA per piece
      eng:   per-piece (cs_engine, combine_engine) chars, 'v'=DVE, 'g'=Pool
      fuse:  per-piece bool — fuse coefA-scale+combine into one DVE
             scalar_tensor_tensor (one ACT mul instead of two, one less hop)
    The goal is that no engine holds a backlog when the last input piece
    lands: the final pieces' chains then drain every engine in parallel.

    hold_stores: the first hold_stores tiles' output stores are deferred
    (scheduler wait-until hint) so they drain the exclusive DMA engines
    while the last tiles compute."""
    if tail_cfg is None:
        tail_cfg = []
    if head_cfg is None:
        head_cfg = []
    f16 = mybir.dt.float16
    f32 = mybir.dt.float32
    tpb = (C * OH) // (P * krp)
    nt = BPC * tpb
    fin = krp * 2 * W
    fout = krp * OW
    assert 112 % krp == 0 and (C * OH) % (P * krp) == 0
    for spec in list(tail_cfg) + list(head_cfg):
        assert sum(spec["plan"]) == krp and all(s >= 3 for s in spec["plan"])
        assert len(spec["eng"]) == len(spec["plan"])

    nc = _SplitWaitsBass()
    x_d = nc.dram_tensor("x", [nt, P, fin], f16, kind="ExternalInput")
    coef_d = nc.dram_tensor("coef", [P, 2 * tpb], f32, kind="ExternalInput")
    out_d = nc.dram_tensor("out", [nt, P, fout], f16, kind="ExternalOutput")

    with TileContext(nc) as tc:
        with (
            tc.tile_pool(name="cpool", bufs=1) as cpool,
            tc.tile_pool(name="xpool", bufs=xbufs) as xpool,
            tc.tile_pool(name="rpool", bufs=rbufs) as rpool,
            tc.tile_pool(name="opool", bufs=obufs) as opool,
            tc.tile_pool(name="hpool", bufs=max(hold_stores, 1)) as hpool,
        ):
            held = []
            pending = []  # deferred Pool combines (emitted after next cs)
            # First x tile load is issued before the coef load so the SP DMA
            # ring starts on the big transfer immediately; coef rides the ACT
            # ring.  Multi-sem waits on the consumers are handled by the
            # _SplitWaitsBass serializer.
            xt0 = xpool.tile([P, fin], f16, tag="xt", name="xt0")
            h_plan = (
                head_cfg[0]["plan"]
                if head_cfg and head_cfg[0].get("split")
                else (krp,)
            )
            f0 = h_plan[0] * 2 * W
            nc.sync.dma_start(xt0[:, :f0], x_d[0][:, :f0])
            coef = cpool.tile([P, 2 * tpb], f32)
            nc.scalar.dma_start(coef, coef_d[:, :])
            coefA = coef[:, :tpb]
            coefM = coef[:, tpb:]
            for i in range(nt):
                tb = i % tpb
                if i < len(head_cfg):
                    spec = head_cfg[i]
                elif i >= nt - len(tail_cfg):
                    spec = tail_cfg[i - (nt - len(tail_cfg))]
                else:
                    spec = None
                plan = spec["plan"] if spec else (krp,)
                split_in = bool(spec and spec.get("split"))

                if i == 0:
                    xt = xt0
                    off = h_plan[0]
                    for seg in h_plan[1:]:
                        fi = seg * 2 * W
                        nc.sync.dma_start(
                            xt[:, off * 2 * W : off * 2 * W + fi],
                            x_d[0][:, off * 2 * W : off * 2 * W + fi],
                        )
                        off += seg
                elif not split_in:
                    xt = xpool.tile([P, fin], f16, tag="xt")
                    nc.sync.dma_start(xt, x_d[i])
                else:
                    xt = xpool.tile([P, fin], f16, tag="xt")
                    off = 0
                    for seg in plan:
                        fi = seg * 2 * W
                        nc.sync.dma_start(
                            xt[:, off * 2 * W : off * 2 * W + fi],
                            x_d[i][:, off * 2 * W : off * 2 * W + fi],
                        )
                        off += seg
                x4 = xt.rearrange("p (s two w) -> p s two w", two=2, w=W)

                off = 0
                for pi, seg in enumerate(plan):
                    sl = slice(off, off + seg)
                    fo = seg * OW
                    ostart = off * OW
                    off += seg
                    if spec:
                        cs_c, cb_c = spec["eng"][pi]
                        fuse = spec.get("fuse", (False,) * len(plan))[pi]
                    else:
                        cs_c = "g"
                        cb_c = (
                            "g"
                            if comb_pool_mod
                            and i % comb_pool_mod == comb_pool_mod - 1
                            else "v"
                        )
                        fuse = False
                    eng = {"v": nc.vector, "g": nc.gpsimd}

                    # Row stage: full-width contiguous fp16 ops -> DVE 2x.
                    # (every rs_pool_mod-th tile's rs runs on Pool: it feeds
                    # cs on the same engine, so no cross-engine hop is added)
                    rs_eng = (
                        nc.gpsimd
                        if rs_pool_mod
                        and not spec
                        and i % rs_pool_mod == rs_pool_mod - 1
                        else nc.vector
                    )
                    rs = rpool.tile([P, seg * W], f16, tag="rs")
                    rs_eng.tensor_add(
                        rs.rearrange("p (s w) -> p s w", w=W),
                        x4[:, sl, 0, :],
                        x4[:, sl, 1, :],
                    )
                    rm = rpool.tile([P, seg * W], f16, tag="rm")
                    nc.vector.tensor_max(
                        rm.rearrange("p (s w) -> p s w", w=W),
                        x4[:, sl, 0, :],
                        x4[:, sl, 1, :],
                    )

                    # Column stage: stride-2 ops, engine per config.
                    rs4 = rs.rearrange("p (s w two) -> p s w two", two=2, w=OW)
                    cs = rpool.tile([P, fo], f16, tag="cs")
                    eng[cs_c].tensor_add(
                        cs.rearrange("p (s w) -> p s w", w=OW),
                        rs4[:, :, :, 0],
                        rs4[:, :, :, 1],
                    )
                    for fn_ in pending:
                        fn_()
                    pending.clear()
                    rm4 = rm.rearrange("p (s w two) -> p s w two", two=2, w=OW)
                    cm = rpool.tile([P, fo], f16, tag="cm")
                    nc.vector.tensor_max(
                        cm.rearrange("p (s w) -> p s w", w=OW),
                        rm4[:, :, :, 0],
                        rm4[:, :, :, 1],
                    )

                    # Scale + combine.  Normal: both scalings on ACT (in
                    # place), contiguous fp16 add on DVE (2x).  fuse: one ACT
                    # scaling + fused DVE scalar_tensor_tensor (one less
                    # dependency hop on the drain path).
                    hold = i < hold_stores
                    pool_ = hpool if hold else opool
                    ot = pool_.tile([P, fo], f16, tag="oth" if hold else "ot")
                    if fuse:
                        nc.scalar.mul(cm, cm, coefM[:, tb : tb + 1])
                        nc.vector.scalar_tensor_tensor(
                            ot,
                            cs,
                            coefA[:, tb : tb + 1],
                            cm,
                            op0=mybir.AluOpType.mult,
                            op1=mybir.AluOpType.add,
                        )
                    else:
                        nc.scalar.mul(cs, cs, coefA[:, tb : tb + 1])
                        nc.scalar.mul(cm, cm, coefM[:, tb : tb + 1])
                        if cb_c == "g" and comb_defer and not spec:
                            dst = out_d[i][:, ostart : ostart + fo]

                            def emit(ot=ot, cs=cs, cm=cm, dst=dst, hold=hold):
                                nc.gpsimd.tensor_add(ot, cs, cm)
                                if hold:
                                    held.append((dst, ot))
                                else:
                                    nc.scalar.dma_start(dst, ot)

                            pending.append(emit)
                            continue
                        eng[cb_c].tensor_add(ot, cs, cm)
                    if hold:
                        held.append((out_d[i][:, ostart : ostart + fo], ot))
                    else:
                        nc.scalar.dma_start(
                            out_d[i][:, ostart : ostart + fo], ot
                        )
            for fn_ in pending:
                fn_()
            pending.clear()
            # Deferred stores: scheduled (via the wait-until hint, which is
            # scheduler-only and not serialized) to land on the ACT ring just
            # after the input stream ends, so they drain the exclusive DMA
            # engines while the last tiles compute.
            for j, (dst, ot) in enumerate(held):
                with tc.tile_wait_until(hold_wait_ms + j * 1e-4):
                    nc.scalar.dma_start(dst, ot)
    nc._variant = dict(krp=krp, tpb=tpb, nt=nt, fin=fin, fout=fout)
    return nc


# current best configuration used by kernel()
BEST = dict(
    krp=14, xbufs=6, rbufs=4, obufs=6, comb_pool_mod=3, hold_stores=0,
    head_cfg=[
        dict(
            plan=(4, 10),
            split=True,
            eng=[("g", "v"), ("g", "v")],
            fuse=(False, False),
        )
    ],
    tail_cfg=[
        dict(
            plan=(8, 3, 3),
            split=True,
            eng=[("g", "g"), ("v", "v"), ("v", "v")],
            fuse=(False, True, True),
        )
    ],
)


def get_nc():
    if not _nc_cache:
        _nc_cache.append(build_variant(**BEST))
    return _nc_cache[0]


def make_coef(w_avg, w_max, krp, tpb):
    # Coefficients stay f32 (scalar operands are exempt from the fp16 rule);
    # (w*w)/4 is an exact exponent shift in fp32.
    wa = np.asarray(w_avg).reshape(C).astype(np.float32)
    wm = np.asarray(w_max).reshape(C).astype(np.float32)
    ca = (wa * wa) / np.float32(4.0)
    cm = wm * wm
    # partition p of tile tb covers channel (tb*P*krp + p*krp) // OH
    chan = (
        np.arange(tpb)[None, :] * P * krp + np.arange(P)[:, None] * krp
    ) // OH  # (P, tpb)
    return np.concatenate([ca[chan], cm[chan]], axis=1).astype(np.float32)


def make_in_maps(x, w_avg, w_max, v):
    coef = make_coef(w_avg, w_max, v["krp"], v["tpb"])
    x = np.asarray(x).astype(np.float16)
    in_maps = []
    for c in range(NCORES):
        xc = np.ascontiguousarray(x[c * BPC : (c + 1) * BPC]).reshape(
            v["nt"], P, v["fin"]
        )
        in_maps.append({"x": xc, "coef": coef})
    return in_maps


def kernel(x, w_avg, w_max):
    nc = get_nc()
    in_maps = make_in_maps(x, w_avg, w_max, nc._variant)
    try:
        res = run_bass_kernel_spmd(nc, in_maps, core_ids=list(range(NCORES)))
    except Exception:
        # A previously-crashed run can leave the device wedged; one retry
        # after it resets is usually enough.
        import time

        time.sleep(5)
        res = run_bass_kernel_spmd(nc, in_maps, core_ids=list(range(NCORES)))
    outs = [
        r["out"].astype(np.float32).reshape(BPC, C, OH, OW) for r in res.results
    ]
    return np.concatenate(outs, axis=0)
